# revision 1
# baseline (speedup 1.0000x reference)
"""GPT-2 small (L=12, D=768, H=12, S=1024, B=2, V=50257) forward pass on 8
Trainium2 NeuronCores via Bass/Tile.

Sharding: data-parallel over batch + vocab-parallel head, zero collectives.
Measured AllReduce cost on this runtime is ~150-250us fixed per call, so any
per-layer collective scheme (24 calls) loses to redundant compute. Instead:
  - cores 0-3 all compute the full 12-layer body for batch 0 (redundantly,
    SPMD-identical), cores 4-7 for batch 1
  - each core then computes its own quarter of the vocab for its batch's
    LM head (12565-ish cols/core, padded 12800) - the only sharded part
The body loops over 4 weight column-slices per layer (g-loop) accumulating
partial sums locally, which keeps every SBUF tile small.

Layout: activations are kept transposed (feature on partitions, tokens on the
free axis) so every dense matmul takes the weight straight from HBM as the
stationary lhsT with no transposes anywhere. Attention scores are computed in
[key, query] layout; softmax uses exp without max-subtraction (scores for this
model/data are bounded well inside fp32 exp range), the key-sum is a
partition_all_reduce, and 1/sum is folded into the PSUM->SBUF copy of the
attention output. Matmul inputs are float32r (full-rate PE, ~1.5e-4 rel err).
"""

import numpy as np

import concourse.bass as bass
import concourse.tile as tile
from concourse import bacc, mybir
from concourse import bass_utils
from concourse.bass_isa import ReduceOp

F32 = mybir.dt.float32
F32R = mybir.dt.float32r
AL = mybir.AluOpType
ACT = mybir.ActivationFunctionType

# model dims
B, S, D, H, DH, F4, V, L = 2, 1024, 768, 12, 64, 3072, 50257, 12
P = 128
KT = D // P            # 6 k-tiles over the model dim
EPS = 1e-5
SCALE = 1.0 / np.sqrt(DH)

# sharding
NCORES = 8
TPG = 4                # weight column-slices per layer (g loop)
HPC = H // TPG         # heads per slice
DL = HPC * DH          # slice attn width 192
FFL = F4 // TPG        # slice ffn width 768
QB = 512               # query block
NQB = S // QB
NKT = S // P           # key tiles
VC = 512               # vocab chunk
VPAD = 12800           # padded per-core vocab slice (25 chunks of 512)
NVC = VPAD // VC
VSLICE = [12565, 12564, 12564, 12564]
VSTART = [0, 12565, 25129, 37693]

L_BODY = L  # overridable before first kernel() call for debugging

_CACHE = {}


def _build():
    nc = bacc.Bacc("TRN2", target_bir_lowering=False, debug=False,
                   num_devices=NCORES)

    def di(name, shape, dt=F32):
        return nc.dram_tensor(name, shape, dt, kind="ExternalInput").ap()

    x0T = di("x0T", [D, S])
    masks = di("masks", [P, TPG, QB])
    wq_s = di("wq_s", [L_BODY, D, D], F32R)
    wk_s = di("wk_s", [L_BODY, D, D], F32R)
    wv_s = di("wv_s", [L_BODY, D, D], F32R)
    wo_s = di("wo_s", [L_BODY, D, D], F32R)
    w1_s = di("w1_s", [L_BODY, D, F4], F32R)
    w2_s = di("w2_s", [L_BODY, F4, D], F32R)
    ln1g = di("ln1g", [L_BODY, P, KT])
    ln1b = di("ln1b", [L_BODY, P, KT])
    ln2g = di("ln2g", [L_BODY, P, KT])
    ln2b = di("ln2b", [L_BODY, P, KT])
    bq_s = di("bq_s", [L_BODY, DH, H])
    bk_s = di("bk_s", [L_BODY, DH, H])
    bv_s = di("bv_s", [L_BODY, TPG, DL])
    bo_s = di("bo_s", [L_BODY, P, KT])
    b1_s = di("b1_s", [L_BODY, TPG, P, KT])
    b2_s = di("b2_s", [L_BODY, P, KT])
    fng = di("fng", [P, KT])
    fnb = di("fnb", [P, KT])
    hw_s = di("hw_s", [D, VPAD], F32R)
    hb_s = di("hb_s", [1, VPAD])
    logits = nc.dram_tensor("logits", [S, VPAD], F32, kind="ExternalOutput").ap()

    with tile.TileContext(nc) as tc:
        with tc.tile_pool(name="persist", bufs=1) as persist, \
             tc.tile_pool(name="slab", bufs=2) as slab, \
             tc.tile_pool(name="wpool", bufs=2) as wpool, \
             tc.tile_pool(name="qk", bufs=2) as qkpool, \
             tc.tile_pool(name="vp", bufs=1) as vpool, \
             tc.tile_pool(name="op", bufs=1) as opool, \
             tc.tile_pool(name="ep", bufs=3) as eppool, \
             tc.tile_pool(name="sums", bufs=2) as sums, \
             tc.tile_pool(name="tmp", bufs=3) as tmp, \
             tc.tile_pool(name="small", bufs=3) as small, \
             tc.tile_pool(name="psA", bufs=4, space="PSUM") as psA, \
             tc.tile_pool(name="psO", bufs=2, space="PSUM") as psO:

            xT = persist.tile([P, KT, S], F32)
            nc.sync.dma_start(xT, x0T.rearrange("(t p) q -> p t q", p=P))
            masks_sb = persist.tile([P, TPG, QB], F32)
            nc.sync.dma_start(masks_sb, masks)

            def layer_norm(g_ap, b_ap, out_dt=F32R):
                """LN over the feature (partition x KT) axis of xT, done per
                query block. Returns a fresh slab tile with the result."""
                g_t = small.tile([P, KT], F32, tag="gain")
                b_t = small.tile([P, KT], F32, tag="gain")
                nc.sync.dma_start(g_t, g_ap)
                nc.sync.dma_start(b_t, b_ap)
                out = slab.tile([P, KT, S], out_dt, tag="slab")
                for qb in range(NQB):
                    qs = slice(qb * QB, (qb + 1) * QB)
                    acc = tmp.tile([P, QB], F32, tag="acc")
                    accsq = tmp.tile([P, QB], F32, tag="acc")
                    sq = tmp.tile([P, QB], F32, tag="acc")
                    nc.vector.tensor_tensor(acc, xT[:, 0, qs], xT[:, 1, qs], AL.add)
                    for kt in range(2, KT):
                        nc.vector.tensor_tensor(acc, acc, xT[:, kt, qs], AL.add)
                    nc.scalar.activation(accsq, xT[:, 0, qs], ACT.Square)
                    for kt in range(1, KT):
                        nc.scalar.activation(sq, xT[:, kt, qs], ACT.Square)
                        nc.vector.tensor_tensor(accsq, accsq, sq, AL.add)
                    nc.gpsimd.partition_all_reduce(acc, acc, P, ReduceOp.add)
                    nc.gpsimd.partition_all_reduce(accsq, accsq, P, ReduceOp.add)
                    # acc -> mean; accsq -> rstd (replicated across partitions)
                    nc.vector.tensor_scalar_mul(acc, acc, 1.0 / D)
                    nc.vector.tensor_tensor(sq, acc, acc, AL.mult)
                    nc.vector.tensor_scalar_mul(accsq, accsq, 1.0 / D)
                    nc.vector.tensor_tensor(accsq, accsq, sq, AL.subtract)
                    nc.vector.tensor_scalar_add(accsq, accsq, EPS)
                    nc.scalar.activation(accsq, accsq, ACT.Sqrt)
                    nc.vector.reciprocal(accsq, accsq)
                    for kt in range(KT):
                        nc.vector.tensor_tensor(sq, xT[:, kt, qs], acc, AL.subtract)
                        nc.vector.tensor_tensor(sq, sq, accsq, AL.mult)
                        nc.vector.tensor_scalar(
                            out[:, kt, qs], sq, g_t[:, kt:kt + 1], b_t[:, kt:kt + 1],
                            AL.mult, AL.add)
                return out

            def add_residual(part, bias_ap):
                nc.vector.tensor_tensor(xT, xT, part, AL.add)
                b_t = small.tile([P, KT], F32, tag="gain")
                nc.sync.dma_start(b_t, bias_ap)
                for kt in range(KT):
                    nc.vector.tensor_scalar_add(
                        xT[:, kt, :], xT[:, kt, :], b_t[:, kt:kt + 1])

            def accum(dst_ap, ps, first):
                if first:
                    nc.vector.tensor_copy(dst_ap, ps)
                else:
                    nc.vector.tensor_tensor(dst_ap, dst_ap, ps, AL.add)

            for l in range(L_BODY):
                # ---- attention ----
                hT = layer_norm(ln1g[l], ln1b[l])
                part = slab.tile([P, KT, S], F32, tag="slab")

                for g in range(TPG):
                    cs = slice(g * DL, (g + 1) * DL)
                    wq_t = wpool.tile([P, KT, DL], F32R, tag="w")
                    nc.sync.dma_start(
                        wq_t, wq_s[l][:, cs].rearrange("(t p) f -> p t f", p=P))
                    wk_t = wpool.tile([P, KT, DL], F32R, tag="w")
                    nc.sync.dma_start(
                        wk_t, wk_s[l][:, cs].rearrange("(t p) f -> p t f", p=P))
                    wv_t = wpool.tile([P, KT, DL], F32R, tag="w")
                    nc.sync.dma_start(
                        wv_t, wv_s[l][:, cs].rearrange("(t p) f -> p t f", p=P))

                    bq_t = small.tile([DH, HPC], F32, tag="bqk")
                    bk_t = small.tile([DH, HPC], F32, tag="bqk")
                    nc.sync.dma_start(bq_t, bq_s[l][:, g * HPC:(g + 1) * HPC])
                    nc.sync.dma_start(bk_t, bk_s[l][:, g * HPC:(g + 1) * HPC])
                    bv_row = small.tile([1, DL], F32, tag="bvr")
                    nc.sync.dma_start(bv_row, bv_s[l, g:g + 1, :])
                    bv_b = small.tile([P, DL], F32, tag="bvb")
                    nc.gpsimd.partition_broadcast(bv_b, bv_row)

                    qT = qkpool.tile([DH, HPC, S], F32R, tag="qk")
                    kTt = qkpool.tile([DH, HPC, S], F32R, tag="qk")
                    for h in range(HPC):
                        for qb in range(NQB):
                            qs = slice(qb * QB, (qb + 1) * QB)
                            q_ps = psO.tile([DH, QB], F32, tag="psO")
                            k_ps = psO.tile([DH, QB], F32, tag="psO")
                            for kt in range(KT):
                                nc.tensor.matmul(
                                    q_ps, wq_t[:, kt, h * DH:(h + 1) * DH],
                                    hT[:, kt, qs], start=kt == 0, stop=kt == KT - 1)
                            nc.vector.tensor_scalar(
                                qT[:, h, qs], q_ps, bq_t[:, h:h + 1], SCALE,
                                AL.add, AL.mult)
                            for kt in range(KT):
                                nc.tensor.matmul(
                                    k_ps, wk_t[:, kt, h * DH:(h + 1) * DH],
                                    hT[:, kt, qs], start=kt == 0, stop=kt == KT - 1)
                            nc.vector.tensor_scalar_add(
                                kTt[:, h, qs], k_ps, bk_t[:, h:h + 1])
                    v_t = vpool.tile([P, NKT, DL], F32R, tag="v")
                    for tc_ in range(NKT):
                        v_ps = psA.tile([P, QB], F32, tag="psA")
                        for kt in range(KT):
                            nc.tensor.matmul(
                                v_ps[:, :DL], hT[:, kt, tc_ * P:(tc_ + 1) * P],
                                wv_t[:, kt, :], start=kt == 0, stop=kt == KT - 1)
                        nc.vector.tensor_tensor(
                            v_t[:, tc_, :], v_ps[:, :DL], bv_b, AL.add)

                    oT = opool.tile([P, 2, S], F32R, tag="o")
                    for h in range(HPC):
                        for qb in range(NQB):
                            qs = slice(qb * QB, (qb + 1) * QB)
                            nkt = 4 * qb + 4
                            o_ps = psO.tile([DH, QB], F32, tag="psO")
                            ssum = sums.tile([P, QB], F32, tag="ssum")
                            for ti in range(nkt):
                                s_ps = psA.tile([P, QB], F32, tag="psA")
                                nc.tensor.matmul(
                                    s_ps, kTt[:, h, ti * P:(ti + 1) * P],
                                    qT[:, h, qs], start=True, stop=True)
                                e_t = eppool.tile([P, QB], F32R, tag="e")
                                nc.scalar.activation(e_t, s_ps, ACT.Exp)
                                r = ti - 4 * qb
                                if r >= 0:
                                    nc.vector.tensor_tensor(
                                        e_t, e_t, masks_sb[:, r, :], AL.mult)
                                if ti == 0:
                                    nc.vector.tensor_copy(ssum, e_t)
                                else:
                                    nc.vector.tensor_tensor(ssum, ssum, e_t, AL.add)
                                nc.tensor.matmul(
                                    o_ps, v_t[:, ti, h * DH:(h + 1) * DH], e_t,
                                    start=ti == 0, stop=ti == nkt - 1)
                            nc.gpsimd.partition_all_reduce(
                                ssum, ssum, P, ReduceOp.add)
                            rcp = sums.tile([DH, QB], F32, tag="rcp")
                            nc.vector.reciprocal(rcp, ssum[:DH, :])
                            nc.vector.tensor_tensor(
                                oT[(h % 2) * DH:(h % 2) * DH + DH, h // 2, qs],
                                o_ps, rcp, AL.mult)

                    wo_t = wpool.tile([P, 2, D], F32R, tag="w")
                    r0 = g * DL
                    nc.sync.dma_start(wo_t[:, 0, :], wo_s[l, r0:r0 + P, :])
                    nc.sync.dma_start(wo_t[0:DL - P, 1, :], wo_s[l, r0 + P:r0 + DL, :])
                    for oc in range(KT):
                        for qb in range(NQB):
                            ps = psA.tile([P, QB], F32, tag="psA")
                            nc.tensor.matmul(
                                ps, wo_t[:, 0, oc * P:(oc + 1) * P],
                                oT[:, 0, qb * QB:(qb + 1) * QB],
                                start=True, stop=False)
                            nc.tensor.matmul(
                                ps, wo_t[0:DH, 1, oc * P:(oc + 1) * P],
                                oT[0:DH, 1, qb * QB:(qb + 1) * QB],
                                start=False, stop=True)
                            accum(part[:, oc, qb * QB:(qb + 1) * QB], ps, g == 0)
                add_residual(part, bo_s[l])

                # ---- ffn ----
                h2T = layer_norm(ln2g[l], ln2b[l])
                part2 = slab.tile([P, KT, S], F32, tag="slab")
                for g in range(TPG):
                    fs = slice(g * FFL, (g + 1) * FFL)
                    w1_t = wpool.tile([P, KT, FFL], F32R, tag="w")
                    nc.sync.dma_start(
                        w1_t, w1_s[l][:, fs].rearrange("(t p) f -> p t f", p=P))
                    w2_t = wpool.tile([P, KT, D], F32R, tag="w")
                    nc.sync.dma_start(
                        w2_t, w2_s[l][fs, :].rearrange("(t p) f -> p t f", p=P))
                    b1_t = small.tile([P, KT], F32, tag="gain")
                    nc.sync.dma_start(b1_t, b1_s[l, g])
                    for qb in range(NQB):
                        qs = slice(qb * QB, (qb + 1) * QB)
                        ffT = vpool.tile([P, KT, QB], F32R, tag="fft")
                        for fc in range(KT):
                            ps = psA.tile([P, QB], F32, tag="psA")
                            for kt in range(KT):
                                nc.tensor.matmul(
                                    ps, w1_t[:, kt, fc * P:(fc + 1) * P],
                                    h2T[:, kt, qs], start=kt == 0, stop=kt == KT - 1)
                            nc.scalar.activation(
                                ffT[:, fc, :], ps, ACT.Gelu,
                                bias=b1_t[:, fc:fc + 1])
                        for oc in range(KT):
                            ps = psA.tile([P, QB], F32, tag="psA")
                            for kt in range(KT):
                                nc.tensor.matmul(
                                    ps, w2_t[:, kt, oc * P:(oc + 1) * P],
                                    ffT[:, kt, :], start=kt == 0, stop=kt == KT - 1)
                            accum(part2[:, oc, qs], ps, g == 0)
                add_residual(part2, b2_s[l])

            # ---- final LN + vocab-sharded head ----
            xfT = layer_norm(fng, fnb)
            for vc in range(NVC):
                vs = slice(vc * VC, (vc + 1) * VC)
                hw_t = wpool.tile([P, KT, VC], F32R, tag="w")
                nc.sync.dma_start(hw_t, hw_s[:, vs].rearrange("(t p) v -> p t v", p=P))
                hb_row = sums.tile([1, VC], F32, tag="rcp")
                nc.sync.dma_start(hb_row, hb_s[:, vs])
                hb_b = vpool.tile([P, VC], F32, tag="hbb")
                nc.gpsimd.partition_broadcast(hb_b, hb_row)
                for tc_ in range(NKT):
                    ps = psA.tile([P, QB], F32, tag="psA")
                    for kt in range(KT):
                        nc.tensor.matmul(
                            ps, xfT[:, kt, tc_ * P:(tc_ + 1) * P],
                            hw_t[:, kt, :], start=kt == 0, stop=kt == KT - 1)
                    lg = eppool.tile([P, VC], F32, tag="e")
                    nc.vector.tensor_tensor(lg, ps, hb_b, AL.add)
                    nc.sync.dma_start(logits[tc_ * P:(tc_ + 1) * P, vs], lg)

    nc.finalize()
    return nc


def _prep_inputs(inputs):
    f = np.ascontiguousarray
    tokens = np.asarray(inputs["tokens"])
    tok_emb = np.asarray(inputs["tok_emb"], np.float32)
    pos_emb = np.asarray(inputs["pos_emb"], np.float32)

    Lb = L_BODY

    def colmajor(a):  # [..., D] -> [..., P, KT] per-partition columns
        return f(a.reshape(*a.shape[:-1], KT, P).swapaxes(-1, -2).astype(np.float32))

    masks = (np.arange(P)[:, None, None] + P * np.arange(TPG)[None, :, None]
             <= np.arange(QB)[None, None, :]).astype(np.float32)

    b1 = np.asarray(inputs["b1"], np.float32)[:Lb]
    base = {
        "masks": masks,
        "wq_s": f(np.asarray(inputs["wq"], np.float32)[:Lb]),
        "wk_s": f(np.asarray(inputs["wk"], np.float32)[:Lb]),
        "wv_s": f(np.asarray(inputs["wv"], np.float32)[:Lb]),
        "wo_s": f(np.asarray(inputs["wo"], np.float32)[:Lb]),
        "w1_s": f(np.asarray(inputs["w1"], np.float32)[:Lb]),
        "w2_s": f(np.asarray(inputs["w2"], np.float32)[:Lb]),
        "ln1g": colmajor(np.asarray(inputs["ln1_g"], np.float32)[:Lb]),
        "ln1b": colmajor(np.asarray(inputs["ln1_b"], np.float32)[:Lb]),
        "ln2g": colmajor(np.asarray(inputs["ln2_g"], np.float32)[:Lb]),
        "ln2b": colmajor(np.asarray(inputs["ln2_b"], np.float32)[:Lb]),
        "bq_s": f(np.asarray(inputs["bq"], np.float32)[:Lb].reshape(Lb, H, DH).swapaxes(1, 2)),
        "bk_s": f(np.asarray(inputs["bk"], np.float32)[:Lb].reshape(Lb, H, DH).swapaxes(1, 2)),
        "bv_s": f(np.asarray(inputs["bv"], np.float32)[:Lb].reshape(Lb, TPG, DL)),
        "bo_s": colmajor(np.asarray(inputs["bo"], np.float32)[:Lb]),
        "b1_s": colmajor(b1.reshape(Lb, TPG, FFL)),
        "b2_s": colmajor(np.asarray(inputs["b2"], np.float32)[:Lb]),
        "fng": colmajor(np.asarray(inputs["fn_g"], np.float32)),
        "fnb": colmajor(np.asarray(inputs["fn_b"], np.float32)),
    }

    head_w = np.asarray(inputs["head_w"], np.float32)
    head_b = np.asarray(inputs["head_b"], np.float32)

    in_maps = []
    for c in range(NCORES):
        b = c // TPG
        g = c % TPG
        v0, vn = VSTART[g], VSLICE[g]
        hw_pad = np.zeros((D, VPAD), np.float32)
        hw_pad[:, :vn] = head_w[:, v0:v0 + vn]
        hb_pad = np.zeros((1, VPAD), np.float32)
        hb_pad[0, :vn] = head_b[v0:v0 + vn]
        x0 = tok_emb[tokens[b]] + pos_emb[:S]
        m = {"x0T": f(x0.T.astype(np.float32)), "hw_s": hw_pad, "hb_s": hb_pad}
        m.update(base)
        in_maps.append(m)
    return in_maps


def _get_nc():
    key = ("nc", L_BODY)
    if key not in _CACHE:
        _CACHE[key] = _build()
    return _CACHE[key]


def kernel(**inputs):
    nc = _get_nc()
    in_maps = _prep_inputs(inputs)
    res = bass_utils.run_bass_kernel_spmd(nc, in_maps, core_ids=list(range(NCORES)))
    out = np.empty((B, S, V), np.float32)
    for c in range(NCORES):
        b, g = c // TPG, c % TPG
        v0, vn = VSTART[g], VSLICE[g]
        out[b, :, v0:v0 + vn] = res.results[c]["logits"][:, :vn]
    return out



# revision 10
# speedup vs baseline: 1.7521x; 1.7521x over previous
"""GPT-2 small (L=12, D=768, H=12, S=1024, B=2, V=50257) forward pass on 8
Trainium2 NeuronCores via Bass/Tile.

Sharding: data-parallel over batch + vocab-parallel head, zero collectives
(AllReduce on this runtime costs ~150-250us fixed per call; 24 calls lose to
redundant compute). Cores 0-3 compute the full body for batch 0, cores 4-7
for batch 1; each core computes a quarter of the vocab head for its batch.

All matmuls run in bf16 (full PE rate at any moving-dim size, half the DMA
and SBUF of f32r), with f32 PSUM accumulation. Weights are host-folded:
  - LN1/LN2/final-LN gain+bias folded into Wq/Wk/Wv/W1/head_w and their
    biases (exact), so layernorm in-kernel is just (x-mean)*rstd.
  - K bias dropped (softmax is invariant to a per-query score shift).
  - V bias folded into the attention-output bias via bo' = bo + bv@Wo
    (softmax rows sum to 1).
  - The 1/sqrt(dh) scale is folded into Wq'/bq'.
Softmax denominator comes free out of the AV matmul: V tiles carry a 65th
ones-column per head, so PSUM row 64 accumulates sum(exp(scores)); exp uses
no max-subtraction (scores bounded for this model).
Layout: activations transposed (feature on partitions, tokens free), so all
dense matmuls take weights straight from HBM as stationary lhsT with no
transposes. Q/K projections are emitted per head-PAIR (stationary 128 wide).
FFN runs in 3 column-thirds with the residual accumulated directly into xT.
"""

import numpy as np
from ml_dtypes import bfloat16

import concourse.bass as bass
import concourse.tile as tile
from concourse import bacc, mybir
from concourse import bass_utils
from concourse.bass_isa import ReduceOp

F32 = mybir.dt.float32
BF16 = mybir.dt.bfloat16
AL = mybir.AluOpType
ACT = mybir.ActivationFunctionType

# model dims
B, S, D, H, DH, F4, V, L = 2, 1024, 768, 12, 64, 3072, 50257, 12
P = 128
KT = D // P            # 6 k-tiles over the model dim
EPS = 1e-5
SCALE = 1.0 / np.sqrt(DH)

# sharding / tiling
NCORES = 8
QB = 512               # query block
NQB = S // QB
NKT = S // P           # key tiles
NPAIR = H // 2         # head pairs
FT = 3                 # ffn thirds
FFC = F4 // FT         # 1024 ffn cols per third
FCT = FFC // P         # 8 fc tiles per third
VC = 512               # vocab chunk
VPAD = 12800           # padded per-core vocab slice (25 chunks of 512)
NVC = VPAD // VC
VSLICE = [12565, 12564, 12564, 12564]
VSTART = [0, 12565, 25129, 37693]

L_BODY = L  # overridable before first kernel() call for debugging

_CACHE = {}


def _build():
    nc = bacc.Bacc("TRN2", target_bir_lowering=False, debug=False,
                   num_devices=NCORES)

    def di(name, shape, dt=F32):
        return nc.dram_tensor(name, shape, dt, kind="ExternalInput").ap()

    x0T = di("x0T", [D, S])
    masks = di("masks", [P, NKT // 2, QB], BF16)
    vones = di("vones", [P, NKT, H, 1], BF16)
    wq_s = di("wq_s", [L_BODY, D, D], BF16)
    wk_s = di("wk_s", [L_BODY, D, D], BF16)
    wv_s = di("wv_s", [L_BODY, D, D], BF16)
    wo_s = di("wo_s", [L_BODY, D, D], BF16)
    w1_s = di("w1_s", [L_BODY, D, F4], BF16)
    w2_s = di("w2_s", [L_BODY, F4, D], BF16)
    bq_s = di("bq_s", [L_BODY, P, NPAIR])
    bo_s = di("bo_s", [L_BODY, P, KT])
    b1_s = di("b1_s", [L_BODY, FT, P, FCT])
    b2_s = di("b2_s", [L_BODY, P, KT])
    hw_s = di("hw_s", [D, VPAD], BF16)
    hb_s = di("hb_s", [1, VPAD])
    logits = nc.dram_tensor("logits", [S, VPAD], BF16, kind="ExternalOutput").ap()

    with tile.TileContext(nc) as tc:
        with tc.tile_pool(name="persist", bufs=1) as persist, \
             tc.tile_pool(name="zp", bufs=2) as zpool, \
             tc.tile_pool(name="vp", bufs=1) as vpool, \
             tc.tile_pool(name="qk", bufs=2) as qkpool, \
             tc.tile_pool(name="op", bufs=2) as opool, \
             tc.tile_pool(name="wp", bufs=1) as wpool, \
             tc.tile_pool(name="fp", bufs=2) as fpool, \
             tc.tile_pool(name="ffp", bufs=2) as ffpool, \
             tc.tile_pool(name="ep", bufs=3) as eppool, \
             tc.tile_pool(name="sums", bufs=2) as sums, \
             tc.tile_pool(name="tmp", bufs=1) as tmp, \
             tc.tile_pool(name="small", bufs=2) as small, \
             tc.tile_pool(name="psA", bufs=4, space="PSUM") as psA, \
             tc.tile_pool(name="psO", bufs=2, space="PSUM") as psO:

            xT = persist.tile([P, KT, S], F32)
            nc.sync.dma_start(xT, x0T.rearrange("(t p) q -> p t q", p=P))
            masks_sb = persist.tile([P, NKT // 2, QB], BF16)
            nc.sync.dma_start(masks_sb, masks)

            # V tiles: per head 64 value dims + a 65th ones column so the AV
            # matmul accumulates sum(exp) into PSUM row 64. Written once.
            v_t = vpool.tile([P, NKT, H, 65], BF16)
            nc.sync.dma_start(v_t[:, :, :, 64:65], vones)

            def layer_norm_qb(z, qb):
                """(x - mean) * rstd over features for one query block.
                Gain/bias are folded into downstream weights on the host."""
                qs = slice(qb * QB, (qb + 1) * QB)
                acc = tmp.tile([P, QB], F32, tag="acc")
                asq = tmp.tile([P, QB], F32, tag="asq")
                t1 = tmp.tile([P, QB], F32, tag="t1")
                t2 = tmp.tile([P, QB], F32, tag="t2")
                nc.vector.tensor_tensor(acc, xT[:, 0, qs], xT[:, 1, qs], AL.add)
                for kt in range(2, KT):
                    nc.vector.tensor_tensor(acc, acc, xT[:, kt, qs], AL.add)
                nc.scalar.activation(asq, xT[:, 0, qs], ACT.Square)
                for kt in range(1, KT):
                    nc.scalar.activation(t1, xT[:, kt, qs], ACT.Square)
                    nc.vector.tensor_tensor(asq, asq, t1, AL.add)
                nc.gpsimd.partition_all_reduce(acc, acc, P, ReduceOp.add)
                nc.gpsimd.partition_all_reduce(asq, asq, P, ReduceOp.add)
                nc.vector.tensor_scalar_mul(acc, acc, 1.0 / D)        # mean
                nc.vector.tensor_tensor(t1, acc, acc, AL.mult)        # mean^2
                nc.vector.tensor_scalar_mul(asq, asq, 1.0 / D)
                nc.vector.tensor_tensor(asq, asq, t1, AL.subtract)    # var
                nc.vector.tensor_scalar_add(asq, asq, EPS)
                nc.scalar.activation(asq, asq, ACT.Sqrt)
                nc.vector.reciprocal(asq, asq)                        # rstd
                nc.vector.tensor_tensor(t2, acc, asq, AL.mult)        # mean*rstd
                for kt in range(KT):
                    nc.vector.tensor_tensor(t1, xT[:, kt, qs], asq, AL.mult)
                    nc.vector.tensor_tensor(z[:, kt, qs], t1, t2, AL.subtract)

            def layer_norm():
                z = zpool.tile([P, KT, S], BF16, tag="z")
                for qb in range(NQB):
                    layer_norm_qb(z, qb)
                return z

            for l in range(L_BODY):
                # ---- attention ----
                z = layer_norm()

                wv_t = wpool.tile([P, KT, D], BF16, tag="wv")
                nc.sync.dma_start(wv_t, wv_s[l].rearrange("(t p) f -> p t f", p=P))
                wq_t = wpool.tile([P, KT, D], BF16, tag="wq")
                nc.sync.dma_start(wq_t, wq_s[l].rearrange("(t p) f -> p t f", p=P))
                wk_t = wpool.tile([P, KT, D], BF16, tag="wk")
                nc.sync.dma_start(wk_t, wk_s[l].rearrange("(t p) f -> p t f", p=P))
                wo_t = wpool.tile([P, KT, D], BF16, tag="wo")
                nc.sync.dma_start(wo_t, wo_s[l].rearrange("(t p) f -> p t f", p=P))
                bq_t = small.tile([P, NPAIR], F32, tag="bq")
                nc.sync.dma_start(bq_t, bq_s[l])

                # V projection: [keys, features] layout, 2 chunks of 384 cols
                for tc_ in range(NKT):
                    for c2 in range(2):
                        ps = psA.tile([P, QB], F32, tag="psA")
                        for kt in range(KT):
                            nc.tensor.matmul(
                                ps[:, :384], z[:, kt, tc_ * P:(tc_ + 1) * P],
                                wv_t[:, kt, c2 * 384:(c2 + 1) * 384],
                                start=kt == 0, stop=kt == KT - 1)
                        nc.scalar.activation(
                            v_t[:, tc_, c2 * 6:(c2 + 1) * 6, 0:64],
                            ps[:, :384].rearrange("p (h d) -> p h d", d=DH),
                            ACT.Copy)

                oTs = []
                for _qb in range(NQB):
                    oT_b = opool.tile([P, KT, QB], BF16, tag="o", name=f"oT{_qb}")
                    oTs.append(oT_b)
                for pair in range(NPAIR):
                    qp = qkpool.tile([P, S], BF16, tag="qp")
                    kp = qkpool.tile([P, S], BF16, tag="kp")
                    for qb in range(NQB):
                        qs = slice(qb * QB, (qb + 1) * QB)
                        psq = psA.tile([P, QB], F32, tag="psA")
                        for kt in range(KT):
                            nc.tensor.matmul(
                                psq, wq_t[:, kt, pair * P:(pair + 1) * P],
                                z[:, kt, qs], start=kt == 0, stop=kt == KT - 1)
                        nc.vector.tensor_scalar_add(
                            qp[:, qs], psq, bq_t[:, pair:pair + 1])
                        psk = psA.tile([P, QB], F32, tag="psA")
                        for kt in range(KT):
                            nc.tensor.matmul(
                                psk, wk_t[:, kt, pair * P:(pair + 1) * P],
                                z[:, kt, qs], start=kt == 0, stop=kt == KT - 1)
                        nc.scalar.activation(kp[:, qs], psk, ACT.Copy)
                    for h2 in range(2):
                        h = 2 * pair + h2
                        po = h2 * DH
                        for qb in range(NQB):
                            qs = slice(qb * QB, (qb + 1) * QB)
                            nkt = 4 * qb + 4
                            o_ps = psO.tile([65, QB], F32, tag="psO")
                            for ti in range(nkt):
                                s_ps = psA.tile([P, QB], F32, tag="psA")
                                nc.tensor.matmul(
                                    s_ps, kp[po:po + DH, ti * P:(ti + 1) * P],
                                    qp[po:po + DH, qs], start=True, stop=True)
                                e_t = eppool.tile([P, QB], BF16, tag="e")
                                nc.scalar.activation(e_t, s_ps, ACT.Exp)
                                r = ti - 4 * qb
                                if r >= 0:
                                    nc.vector.tensor_tensor(
                                        e_t, e_t, masks_sb[:, r, :], AL.mult)
                                nc.tensor.matmul(
                                    o_ps, v_t[:, ti, h, :], e_t,
                                    start=ti == 0, stop=ti == nkt - 1)
                            rcp1 = sums.tile([1, QB], F32, tag="rcp1", bufs=1)
                            nc.vector.reciprocal(rcp1, o_ps[64:65, :])
                            rcp = sums.tile([DH, QB], F32, tag="rcp")
                            nc.gpsimd.partition_broadcast(rcp, rcp1)
                            nc.vector.tensor_tensor(
                                oTs[qb][po:po + DH, pair, :],
                                o_ps[0:64, :], rcp, AL.mult)

                bo_t = small.tile([P, KT], F32, tag="bias")
                nc.sync.dma_start(bo_t, bo_s[l])
                for qb in range(NQB):
                    qs = slice(qb * QB, (qb + 1) * QB)
                    for oc in range(KT):
                        ps = psA.tile([P, QB], F32, tag="psA")
                        for kt in range(KT):
                            nc.tensor.matmul(
                                ps, wo_t[:, kt, oc * P:(oc + 1) * P],
                                oTs[qb][:, kt, :], start=kt == 0, stop=kt == KT - 1)
                        nc.vector.tensor_tensor(xT[:, oc, qs], xT[:, oc, qs], ps, AL.add)
                        nc.vector.tensor_scalar_add(
                            xT[:, oc, qs], xT[:, oc, qs], bo_t[:, oc:oc + 1])

                # ---- ffn ----
                z2 = layer_norm()
                b2_t = small.tile([P, KT], F32, tag="bias")
                nc.sync.dma_start(b2_t, b2_s[l])
                for g in range(FT):
                    w1_t = fpool.tile([P, KT, FFC], BF16, tag="w1")
                    nc.sync.dma_start(
                        w1_t, w1_s[l][:, g * FFC:(g + 1) * FFC]
                        .rearrange("(t p) f -> p t f", p=P))
                    w2_t = fpool.tile([P, FCT, D], BF16, tag="w2")
                    nc.sync.dma_start(
                        w2_t, w2_s[l][g * FFC:(g + 1) * FFC, :]
                        .rearrange("(t p) f -> p t f", p=P))
                    b1_t = small.tile([P, FCT], F32, tag="b1")
                    nc.sync.dma_start(b1_t, b1_s[l, g])
                    for qb in range(NQB):
                        qs = slice(qb * QB, (qb + 1) * QB)
                        ffT = ffpool.tile([P, FCT, QB], BF16, tag="ff")
                        for fc in range(FCT):
                            ps = psA.tile([P, QB], F32, tag="psA")
                            for kt in range(KT):
                                nc.tensor.matmul(
                                    ps, w1_t[:, kt, fc * P:(fc + 1) * P],
                                    z2[:, kt, qs], start=kt == 0, stop=kt == KT - 1)
                            nc.scalar.activation(
                                ffT[:, fc, :], ps, ACT.Gelu,
                                bias=b1_t[:, fc:fc + 1])
                        for oc in range(KT):
                            ps = psA.tile([P, QB], F32, tag="psA")
                            for kt in range(FCT):
                                nc.tensor.matmul(
                                    ps, w2_t[:, kt, oc * P:(oc + 1) * P],
                                    ffT[:, kt, :], start=kt == 0, stop=kt == FCT - 1)
                            nc.vector.tensor_tensor(
                                xT[:, oc, qs], xT[:, oc, qs], ps, AL.add)
                            if g == FT - 1:
                                nc.vector.tensor_scalar_add(
                                    xT[:, oc, qs], xT[:, oc, qs], b2_t[:, oc:oc + 1])

            # ---- final LN (fn gain/bias folded into head) + vocab head ----
            zf = layer_norm()
            for vc in range(NVC):
                vs = slice(vc * VC, (vc + 1) * VC)
                hw_t = fpool.tile([P, KT, VC], BF16, tag="w1")
                nc.sync.dma_start(hw_t, hw_s[:, vs].rearrange("(t p) v -> p t v", p=P))
                hb_row = sums.tile([1, VC], F32, tag="rcp1", bufs=1)
                nc.sync.dma_start(hb_row, hb_s[:, vs])
                hb_b = vpool.tile([P, VC], F32, tag="hbb")
                nc.gpsimd.partition_broadcast(hb_b, hb_row)
                for tc_ in range(NKT):
                    ps = psA.tile([P, QB], F32, tag="psA")
                    for kt in range(KT):
                        nc.tensor.matmul(
                            ps[:, :VC], zf[:, kt, tc_ * P:(tc_ + 1) * P],
                            hw_t[:, kt, :], start=kt == 0, stop=kt == KT - 1)
                    lg = eppool.tile([P, VC], BF16, tag="lg", bufs=2)
                    nc.vector.tensor_tensor(lg, ps[:, :VC], hb_b, AL.add)
                    nc.sync.dma_start(logits[tc_ * P:(tc_ + 1) * P, vs], lg)

    nc.finalize()
    return nc


def _prep_inputs(inputs):
    f = np.ascontiguousarray
    tokens = np.asarray(inputs["tokens"])
    tok_emb = np.asarray(inputs["tok_emb"], np.float32)
    pos_emb = np.asarray(inputs["pos_emb"], np.float32)

    Lb = L_BODY

    def colmajor(a):  # [..., D] -> [..., P, KT] per-partition columns
        return f(a.reshape(*a.shape[:-1], KT, P).swapaxes(-1, -2).astype(np.float32))

    def bf(a):
        return f(np.asarray(a).astype(bfloat16))

    masks = (np.arange(P)[:, None, None] + P * np.arange(NKT // 2)[None, :, None]
             <= np.arange(QB)[None, None, :]).astype(bfloat16)

    g1 = np.asarray(inputs["ln1_g"], np.float64)[:Lb]   # [L, D]
    c1 = np.asarray(inputs["ln1_b"], np.float64)[:Lb]
    g2 = np.asarray(inputs["ln2_g"], np.float64)[:Lb]
    c2 = np.asarray(inputs["ln2_b"], np.float64)[:Lb]
    wq = np.asarray(inputs["wq"], np.float64)[:Lb]      # [L, D, D]
    wk = np.asarray(inputs["wk"], np.float64)[:Lb]
    wv = np.asarray(inputs["wv"], np.float64)[:Lb]
    wo = np.asarray(inputs["wo"], np.float64)[:Lb]
    w1 = np.asarray(inputs["w1"], np.float64)[:Lb]
    w2 = np.asarray(inputs["w2"], np.float64)[:Lb]
    bq = np.asarray(inputs["bq"], np.float64)[:Lb]
    bv = np.asarray(inputs["bv"], np.float64)[:Lb]
    bo = np.asarray(inputs["bo"], np.float64)[:Lb]
    b1 = np.asarray(inputs["b1"], np.float64)[:Lb]
    b2 = np.asarray(inputs["b2"], np.float64)[:Lb]
    fng = np.asarray(inputs["fn_g"], np.float64)
    fnb = np.asarray(inputs["fn_b"], np.float64)
    head_w = np.asarray(inputs["head_w"], np.float64)
    head_b = np.asarray(inputs["head_b"], np.float64)

    # fold LN1 gain/bias into Wq/Wk/Wv (+ 1/sqrt(dh) scale into Wq/bq);
    # drop K bias (softmax shift-invariance); fold V bias into bo via Wo.
    wq_f = g1[:, :, None] * wq * SCALE
    bq_f = (np.einsum("ld,ldf->lf", c1, wq) + bq) * SCALE
    wk_f = g1[:, :, None] * wk
    wv_f = g1[:, :, None] * wv
    bv_f = np.einsum("ld,ldf->lf", c1, wv) + bv
    bo_f = np.einsum("ld,ldf->lf", bv_f, wo) + bo
    # fold LN2 gain/bias into W1/b1
    w1_f = g2[:, :, None] * w1
    b1_f = np.einsum("ld,ldf->lf", c2, w1) + b1
    # fold final LN gain/bias into head
    hw_f = fng[:, None] * head_w
    hb_f = fnb @ head_w + head_b

    # bq: f = pair*128 + (h%2)*64 + dh -> sbuf [128, NPAIR]
    bq_pairs = f(bq_f.reshape(Lb, NPAIR, P).swapaxes(1, 2).astype(np.float32))
    b1_thirds = f(b1_f.reshape(Lb, FT, FCT, P).swapaxes(2, 3).astype(np.float32))

    base = {
        "masks": masks,
        "vones": np.ones((P, NKT, H, 1), bfloat16),
        "wq_s": bf(wq_f), "wk_s": bf(wk_f), "wv_s": bf(wv_f), "wo_s": bf(wo),
        "w1_s": bf(w1_f), "w2_s": bf(w2),
        "bq_s": bq_pairs,
        "bo_s": colmajor(bo_f),
        "b1_s": b1_thirds,
        "b2_s": colmajor(b2),
    }

    in_maps = []
    for c in range(NCORES):
        b = c // 4
        g = c % 4
        v0, vn = VSTART[g], VSLICE[g]
        hw_pad = np.zeros((D, VPAD), np.float64)
        hw_pad[:, :vn] = hw_f[:, v0:v0 + vn]
        hb_pad = np.zeros((1, VPAD), np.float32)
        hb_pad[0, :vn] = hb_f[v0:v0 + vn]
        x0 = tok_emb[tokens[b]] + pos_emb[:S]
        m = {"x0T": f(x0.T.astype(np.float32)), "hw_s": bf(hw_pad),
             "hb_s": hb_pad}
        m.update(base)
        in_maps.append(m)
    return in_maps


def _get_nc():
    key = ("nc", L_BODY)
    if key not in _CACHE:
        _CACHE[key] = _build()
    return _CACHE[key]


def kernel(**inputs):
    nc = _get_nc()
    in_maps = _prep_inputs(inputs)
    res = bass_utils.run_bass_kernel_spmd(nc, in_maps, core_ids=list(range(NCORES)))
    out = np.empty((B, S, V), np.float32)
    for c in range(NCORES):
        b, g = c // 4, c % 4
        v0, vn = VSTART[g], VSLICE[g]
        out[b, :, v0:v0 + vn] = np.asarray(res.results[c]["logits"])[:, :vn].astype(np.float32)
    return out


# revision 39
# speedup vs baseline: 1.9427x; 1.1088x over previous
"""GPT-2 small (L=12, D=768, H=12, S=1024, B=2, V=50257) forward pass on 8
Trainium2 NeuronCores via Bass/Tile.

Sharding: data-parallel over batch + vocab-parallel head, zero collectives
(AllReduce on this runtime costs ~150-250us fixed per call; 24 calls lose to
redundant compute). Cores 0-3 compute the full body for batch 0, cores 4-7
for batch 1; each core computes a quarter of the vocab head for its batch.

All matmuls run in bf16 (full PE rate at any moving-dim size, half the DMA
and SBUF of f32r), with f32 PSUM accumulation. Weights are host-folded:
  - LN1/LN2/final-LN gain+bias folded into Wq/Wk/Wv/W1/head_w and their
    biases (exact), so layernorm in-kernel is just (x-mean)*rstd.
  - K bias dropped (softmax is invariant to a per-query score shift).
  - V bias folded into the attention-output bias via bo' = bo + bv@Wo
    (softmax rows sum to 1).
  - The 1/sqrt(dh) scale is folded into Wq'/bq'.
Softmax denominator comes free out of the AV matmul: V tiles carry a 65th
ones-column per head, so PSUM row 64 accumulates sum(exp(scores)); exp uses
no max-subtraction (scores bounded for this model).
Layout: activations transposed (feature on partitions, tokens free), so all
dense matmuls take weights straight from HBM as stationary lhsT with no
transposes. Q/K projections are emitted per head-PAIR (stationary 128 wide).
FFN runs in 3 column-thirds with the residual accumulated directly into xT.
"""

import numpy as np
from ml_dtypes import bfloat16

import concourse.bass as bass
import concourse.tile as tile
from concourse import bacc, mybir
from concourse import bass_utils
from concourse.bass_isa import ReduceOp

F32 = mybir.dt.float32
BF16 = mybir.dt.bfloat16
AL = mybir.AluOpType
ACT = mybir.ActivationFunctionType

# model dims
B, S, D, H, DH, F4, V, L = 2, 1024, 768, 12, 64, 3072, 50257, 12
P = 128
KT = D // P            # 6 k-tiles over the model dim
EPS = 1e-5
SCALE = 1.0 / np.sqrt(DH)

# sharding / tiling
NCORES = 8
QB = 512               # query block
NQB = S // QB
NKT = S // P           # key tiles
NPAIR = H // 2         # head pairs
FT = 6                 # ffn column slices
FFC = F4 // FT         # 512 ffn cols per slice
FCT = FFC // P         # 4 fc tiles per slice
VC = 512               # vocab chunk
VPAD = 12800           # padded per-core vocab slice (25 chunks of 512)
NVC = VPAD // VC
VSLICE = [12565, 12564, 12564, 12564]
VSTART = [0, 12565, 25129, 37693]

L_BODY = L  # overridable before first kernel() call for debugging

_CACHE = {}


def _build():
    nc = bacc.Bacc("TRN2", target_bir_lowering=False, debug=False,
                   num_devices=NCORES)

    # register EPS as a const AP so activation(bias=EPS) resolves
    eps_tensor = nc.alloc_sbuf_tensor(f"const-float32-{EPS}", [P, 1], F32)
    nc.gpsimd.memset(eps_tensor.ap(), EPS)
    nc.const_aps.aps[(F32, EPS)] = eps_tensor.ap()

    def di(name, shape, dt=F32):
        return nc.dram_tensor(name, shape, dt, kind="ExternalInput").ap()

    x0T = di("x0T", [D, S])
    masks = di("masks", [P, NKT // 2, QB], BF16)
    vones = di("vones", [P, NKT, H, 1], BF16)
    wq_s = di("wq_s", [L_BODY, D, D], BF16)
    wk_s = di("wk_s", [L_BODY, D, D], BF16)
    wv_s = di("wv_s", [L_BODY, D, D], BF16)
    wo_s = di("wo_s", [L_BODY, D, D], BF16)
    w1_s = di("w1_s", [L_BODY, D, F4], BF16)
    w2_s = di("w2_s", [L_BODY, F4, D], BF16)
    bq_s = di("bq_s", [L_BODY, P, NPAIR])
    bo_s = di("bo_s", [L_BODY, P, KT])
    b1_s = di("b1_s", [L_BODY, FT, P, FCT])
    b2_s = di("b2_s", [L_BODY, P, KT])
    hw_s = di("hw_s", [D, VPAD], BF16)
    logits = nc.dram_tensor("logits", [S, VPAD], BF16, kind="ExternalOutput").ap()

    with tile.TileContext(nc) as tc:
        with tc.tile_pool(name="persist", bufs=1) as persist, \
             tc.tile_pool(name="zp", bufs=2) as zpool, \
             tc.tile_pool(name="vp", bufs=1) as vpool, \
             tc.tile_pool(name="qk", bufs=2) as qkpool, \
             tc.tile_pool(name="op", bufs=2) as opool, \
             tc.tile_pool(name="wp", bufs=1) as wpool, \
             tc.tile_pool(name="fp", bufs=2) as fpool, \
             tc.tile_pool(name="ffp", bufs=2) as ffpool, \
             tc.tile_pool(name="ep", bufs=3) as eppool, \
             tc.tile_pool(name="sums", bufs=2) as sums, \
             tc.tile_pool(name="tmp", bufs=1) as tmp, \
             tc.tile_pool(name="small", bufs=2) as small, \
             tc.tile_pool(name="psA", bufs=2, space="PSUM") as psA, \
             tc.tile_pool(name="psS", bufs=2, space="PSUM") as psS, \
             tc.tile_pool(name="psO", bufs=2, space="PSUM") as psO:

            xT = persist.tile([P, KT, S], F32)
            nc.sync.dma_start(xT, x0T.rearrange("(t p) q -> p t q", p=P))
            masks_sb = persist.tile([P, NKT // 2, QB], BF16)
            nc.sync.dma_start(masks_sb, masks)

            # V tiles: per head 64 value dims + a 65th ones column so the AV
            # matmul accumulates sum(exp) into PSUM row 64. Written once.
            v_t = vpool.tile([P, NKT, H, 65], BF16)
            nc.sync.dma_start(v_t[:, :, :, 64:65], vones)

            def ln_stat_tiles():
                st = tmp.tile([P, 2, QB], F32, tag="st")
                return st

            def ln_stat_oc(st, qb, oc):
                """Accumulate LN sums for one feature tile of xT, emitted
                right after that tile's residual add so the stats pipeline
                with the producing matmul loop. st[:,0]=sum, st[:,1]=sumsq."""
                qs = slice(qb * QB, (qb + 1) * QB)
                tsq = tmp.tile([P, QB], F32, tag="t3")
                if oc == 0:
                    nc.vector.tensor_copy(st[:, 0, :], xT[:, 0, qs])
                    nc.scalar.activation(st[:, 1, :], xT[:, 0, qs], ACT.Square)
                else:
                    nc.vector.tensor_tensor(st[:, 0, :], st[:, 0, :],
                                            xT[:, oc, qs], AL.add)
                    nc.scalar.activation(tsq, xT[:, oc, qs], ACT.Square)
                    nc.vector.tensor_tensor(st[:, 1, :], st[:, 1, :], tsq, AL.add)

            def ln_finish(st, z, qb):
                """(x - mean) * rstd from accumulated sums. Gain/bias are
                folded into downstream weights on the host. The normalize is
                split across DVE and Pool to halve its serial tail."""
                qs = slice(qb * QB, (qb + 1) * QB)
                t1 = tmp.tile([P, QB], F32, tag="t1")
                t2 = tmp.tile([P, QB], F32, tag="t2")
                t3 = tmp.tile([P, QB], F32, tag="t3")
                t4 = tmp.tile([P, QB], F32, tag="t4")
                # one fused partition reduce for both sums
                nc.gpsimd.partition_all_reduce(st, st, P, ReduceOp.add)
                # t1 = mean^2 * D = (acc/D)*acc
                nc.vector.scalar_tensor_tensor(
                    t1, st[:, 0, :], 1.0 / D, st[:, 0, :], AL.mult, AL.mult)
                nc.vector.tensor_tensor(t1, st[:, 1, :], t1, AL.subtract)
                # sigma = sqrt((asq - m^2 D)/D + eps), then rstd = 1/sigma
                nc.scalar.activation(t1, t1, ACT.Sqrt, scale=1.0 / D, bias=EPS)
                nc.vector.reciprocal(t1, t1)                          # rstd
                # t2 = mean * rstd
                nc.vector.scalar_tensor_tensor(
                    t2, st[:, 0, :], 1.0 / D, t1, AL.mult, AL.mult)
                for kt in range(3):
                    nc.vector.tensor_tensor(t3, xT[:, kt, qs], t1, AL.mult)
                    nc.vector.tensor_tensor(z[:, kt, qs], t3, t2, AL.subtract)
                for kt in range(3, KT):
                    nc.gpsimd.tensor_tensor(t4, xT[:, kt, qs], t1, AL.mult)
                    nc.gpsimd.tensor_tensor(z[:, kt, qs], t4, t2, AL.subtract)

            def layer_norm():
                z = zpool.tile([P, KT, S], BF16, tag="z")
                for qb in range(NQB):
                    st = ln_stat_tiles()
                    for oc in range(KT):
                        ln_stat_oc(st, qb, oc)
                    ln_finish(st, z, qb)
                return z

            z = layer_norm()
            for l in range(L_BODY):
                # ---- attention ----
                wv_t = wpool.tile([P, KT, D], BF16, tag="wv")
                nc.sync.dma_start(wv_t, wv_s[l].rearrange("(t p) f -> p t f", p=P))
                wq_t = wpool.tile([P, KT, D], BF16, tag="wq")
                nc.sync.dma_start(wq_t, wq_s[l].rearrange("(t p) f -> p t f", p=P))
                wk_t = wpool.tile([P, KT, D], BF16, tag="wk")
                nc.sync.dma_start(wk_t, wk_s[l].rearrange("(t p) f -> p t f", p=P))
                wo_t = wpool.tile([P, KT, D], BF16, tag="wo")
                nc.sync.dma_start(wo_t, wo_s[l].rearrange("(t p) f -> p t f", p=P))
                bq_t = small.tile([P, NPAIR], F32, tag="bq")
                nc.sync.dma_start(bq_t, bq_s[l])

                # V projection: [keys, features] layout, 2 chunks of 384 cols
                def v_proj(tcs):
                    for tc_ in tcs:
                        for c2 in range(2):
                            ps = psA.tile([P, QB], F32, tag="psA")
                            for kt in range(KT):
                                nc.tensor.matmul(
                                    ps[:, :384], z[:, kt, tc_ * P:(tc_ + 1) * P],
                                    wv_t[:, kt, c2 * 384:(c2 + 1) * 384],
                                    start=kt == 0, stop=kt == KT - 1)
                            nc.vector.tensor_copy(
                                v_t[:, tc_, c2 * 6:(c2 + 1) * 6, 0:64],
                                ps[:, :384].rearrange("p (h d) -> p h d", d=DH))

                # keys 0-511 now; keys 512-1023 deferred until the first
                # pair's qb0 attention is emitted, so the PE has qb0-only
                # work while the layer-boundary LN of qb1 finishes
                v_proj(range(0, NKT // 2))

                oTs = []
                for _qb in range(NQB):
                    oT_b = opool.tile([P, KT, QB], BF16, tag="o", name=f"oT{_qb}")
                    oTs.append(oT_b)

                def qk_proj(pair, qp, kp, qb):
                    qs = slice(qb * QB, (qb + 1) * QB)
                    pqk = psS.tile([P, 2, QB], F32, tag="psS")
                    for kt in range(KT):
                        nc.tensor.matmul(
                            pqk[:, 0, :], wq_t[:, kt, pair * P:(pair + 1) * P],
                            z[:, kt, qs], start=kt == 0, stop=kt == KT - 1)
                    for kt in range(KT):
                        nc.tensor.matmul(
                            pqk[:, 1, :], wk_t[:, kt, pair * P:(pair + 1) * P],
                            z[:, kt, qs], start=kt == 0, stop=kt == KT - 1)
                    nc.vector.tensor_scalar_add(
                        qp[:, qs], pqk[:, 0, :], bq_t[:, pair:pair + 1])
                    nc.vector.tensor_copy(kp[:, qs], pqk[:, 1, :])

                def attn_unit(pair, qp, kp, qb):
                    qs = slice(qb * QB, (qb + 1) * QB)
                    nkt = 4 * qb + 4
                    o_ps = []
                    for h2 in range(2):
                        ops_h = psO.tile([65, QB], F32, tag="psO",
                                         name=f"ops{h2}")
                        o_ps.append(ops_h)
                    # software pipeline: AV of tile ti-1 is emitted after
                    # the scores of tile ti, so exp/mask latency is hidden
                    prev = None
                    for ti in range(nkt):
                        s_ps = psS.tile([P, 2, QB], F32, tag="psS")
                        for h2 in range(2):
                            po = h2 * DH
                            nc.tensor.matmul(
                                s_ps[:, h2, :], kp[po:po + DH, ti * P:(ti + 1) * P],
                                qp[po:po + DH, qs], start=True, stop=True)
                        e_t = eppool.tile([P, 2, QB], BF16, tag="e", bufs=4)
                        nc.scalar.activation(e_t, s_ps, ACT.Exp)
                        r = ti - 4 * qb
                        if r >= 0:
                            nc.vector.tensor_tensor(
                                e_t[:, 0, :], e_t[:, 0, :], masks_sb[:, r, :], AL.mult)
                            nc.vector.tensor_tensor(
                                e_t[:, 1, :], e_t[:, 1, :], masks_sb[:, r, :], AL.mult)
                        if prev is not None:
                            for h2 in range(2):
                                nc.tensor.matmul(
                                    o_ps[h2], v_t[:, ti - 1, 2 * pair + h2, :],
                                    prev[:, h2, :], start=ti - 1 == 0, stop=False)
                        prev = e_t
                    for h2 in range(2):
                        nc.tensor.matmul(
                            o_ps[h2], v_t[:, nkt - 1, 2 * pair + h2, :],
                            prev[:, h2, :], start=nkt == 1, stop=True)
                    for h2 in range(2):
                        po = h2 * DH
                        rcp1 = sums.tile([1, QB], F32, tag="rcp1", bufs=1)
                        nc.vector.tensor_copy(rcp1, o_ps[h2][64:65, :])
                        rcp = sums.tile([DH, QB], F32, tag="rcp", bufs=2)
                        nc.gpsimd.partition_broadcast(rcp, rcp1)
                        nc.vector.tensor_tensor(
                            oTs[qb][po:po + DH, pair, :],
                            o_ps[h2][0:64, :], rcp, AL.divide)

                # pair-level software pipeline: projections of pair p are
                # emitted before the attention of pair p-1, so the psum->sbuf
                # copies always have a full projection block of PE work to
                # complete behind
                pk_prev = None
                for pair in range(NPAIR):
                    qp = qkpool.tile([P, S], BF16, tag="qp")
                    kp = qkpool.tile([P, S], BF16, tag="kp")
                    qk_proj(pair, qp, kp, 0)
                    qk_proj(pair, qp, kp, 1)
                    if pair == 0:
                        v_proj(range(NKT // 2, NKT))
                    if pk_prev is not None:
                        pp, pqp, pkp = pk_prev
                        attn_unit(pp, pqp, pkp, 0)
                        attn_unit(pp, pqp, pkp, 1)
                    pk_prev = (pair, qp, kp)
                pp, pqp, pkp = pk_prev
                attn_unit(pp, pqp, pkp, 0)
                attn_unit(pp, pqp, pkp, 1)

                bo_t = small.tile([P, KT], F32, tag="bias")
                nc.sync.dma_start(bo_t, bo_s[l])
                z2 = zpool.tile([P, KT, S], BF16, tag="z", name="z2")
                for qb in range(NQB):
                    qs = slice(qb * QB, (qb + 1) * QB)
                    st = ln_stat_tiles()
                    for op_ in range(KT // 2):
                        ps = psS.tile([P, 2, QB], F32, tag="psS")
                        for half in range(2):
                            oc = 2 * op_ + half
                            for kt in range(KT):
                                nc.tensor.matmul(
                                    ps[:, half, :], wo_t[:, kt, oc * P:(oc + 1) * P],
                                    oTs[qb][:, kt, :], start=kt == 0, stop=kt == KT - 1)
                        nc.vector.tensor_tensor(
                            xT[:, 2 * op_:2 * op_ + 2, qs],
                            xT[:, 2 * op_:2 * op_ + 2, qs], ps, AL.add)
                        for half in range(2):
                            oc = 2 * op_ + half
                            nc.vector.tensor_scalar_add(
                                xT[:, oc, qs], xT[:, oc, qs], bo_t[:, oc:oc + 1])
                            # LN2 stats pipeline with the O-proj oc loop
                            ln_stat_oc(st, qb, oc)
                    ln_finish(st, z2, qb)

                # ---- ffn (qb outer so LN of the next layer overlaps) ----
                b2_t = small.tile([P, KT], F32, tag="bias")
                nc.sync.dma_start(b2_t, b2_s[l])
                z_next = zpool.tile([P, KT, S], BF16, tag="z", name="z_next")

                def ffn_slice(qb, g, z2, b2_t):
                    qs = slice(qb * QB, (qb + 1) * QB)
                    w1_t = fpool.tile([P, KT, FFC], BF16, tag="w1")
                    nc.sync.dma_start(
                        w1_t, w1_s[l][:, g * FFC:(g + 1) * FFC]
                        .rearrange("(t p) f -> p t f", p=P))
                    w2_t = fpool.tile([P, FCT, D], BF16, tag="w2")
                    nc.sync.dma_start(
                        w2_t, w2_s[l][g * FFC:(g + 1) * FFC, :]
                        .rearrange("(t p) f -> p t f", p=P))
                    b1_t = small.tile([P, FCT], F32, tag="b1")
                    nc.sync.dma_start(b1_t, b1_s[l, g])
                    ffT = ffpool.tile([P, FCT, QB], BF16, tag="ff")
                    for fc in range(FCT):
                        ps = psA.tile([P, QB], F32, tag="psA")
                        for kt in range(KT):
                            nc.tensor.matmul(
                                ps, w1_t[:, kt, fc * P:(fc + 1) * P],
                                z2[:, kt, qs], start=kt == 0, stop=kt == KT - 1)
                        nc.scalar.activation(
                            ffT[:, fc, :], ps, ACT.Gelu,
                            bias=b1_t[:, fc:fc + 1])
                    st = ln_stat_tiles() if g == FT - 1 else None
                    for op_ in range(KT // 2):
                        ps = psS.tile([P, 2, QB], F32, tag="psS")
                        for half in range(2):
                            oc = 2 * op_ + half
                            for kt in range(FCT):
                                nc.tensor.matmul(
                                    ps[:, half, :], w2_t[:, kt, oc * P:(oc + 1) * P],
                                    ffT[:, kt, :], start=kt == 0, stop=kt == FCT - 1)
                        nc.vector.tensor_tensor(
                            xT[:, 2 * op_:2 * op_ + 2, qs],
                            xT[:, 2 * op_:2 * op_ + 2, qs], ps, AL.add)
                        if g == FT - 1:
                            for half in range(2):
                                oc = 2 * op_ + half
                                nc.vector.tensor_scalar_add(
                                    xT[:, oc, qs], xT[:, oc, qs], b2_t[:, oc:oc + 1])
                                # next layer's LN stats pipeline with FFN2
                                ln_stat_oc(st, qb, oc)
                    return st

                for qb in range(NQB):
                    for g in range(FT):
                        st = ffn_slice(qb, g, z2, b2_t) or st
                    # LN for the next layer (or the folded final LN) for this
                    # query block, overlapped with the other block's FFN
                    ln_finish(st, z_next, qb)
                z = z_next

            # ---- vocab head (final LN folded: z is the final-LN output;
            # head bias is added on the host) ----
            zf = z
            for vc in range(NVC):
                vs = slice(vc * VC, (vc + 1) * VC)
                hw_t = fpool.tile([P, KT, VC], BF16, tag="w1")
                nc.sync.dma_start(hw_t, hw_s[:, vs].rearrange("(t p) v -> p t v", p=P))
                for tc_ in range(NKT):
                    ps = psA.tile([P, QB], F32, tag="psA")
                    for kt in range(KT):
                        nc.tensor.matmul(
                            ps[:, :VC], zf[:, kt, tc_ * P:(tc_ + 1) * P],
                            hw_t[:, kt, :], start=kt == 0, stop=kt == KT - 1)
                    lg = eppool.tile([P, VC], BF16, tag="lg", bufs=3)
                    nc.scalar.activation(lg, ps[:, :VC], ACT.Copy)
                    # Pool-issued DMA: keeps logits writes off the SP queue so
                    # they don't head-of-line-block the next hw_t load
                    nc.gpsimd.dma_start(logits[tc_ * P:(tc_ + 1) * P, vs], lg)

    nc.finalize()
    return nc


def _prep_inputs(inputs):
    f = np.ascontiguousarray
    tokens = np.asarray(inputs["tokens"])
    tok_emb = np.asarray(inputs["tok_emb"], np.float32)
    pos_emb = np.asarray(inputs["pos_emb"], np.float32)

    Lb = L_BODY

    def colmajor(a):  # [..., D] -> [..., P, KT] per-partition columns
        return f(a.reshape(*a.shape[:-1], KT, P).swapaxes(-1, -2).astype(np.float32))

    def bf(a):
        return f(np.asarray(a).astype(bfloat16))

    masks = (np.arange(P)[:, None, None] + P * np.arange(NKT // 2)[None, :, None]
             <= np.arange(QB)[None, None, :]).astype(bfloat16)

    g1 = np.asarray(inputs["ln1_g"], np.float64)[:Lb]   # [L, D]
    c1 = np.asarray(inputs["ln1_b"], np.float64)[:Lb]
    g2 = np.asarray(inputs["ln2_g"], np.float64)[:Lb]
    c2 = np.asarray(inputs["ln2_b"], np.float64)[:Lb]
    wq = np.asarray(inputs["wq"], np.float64)[:Lb]      # [L, D, D]
    wk = np.asarray(inputs["wk"], np.float64)[:Lb]
    wv = np.asarray(inputs["wv"], np.float64)[:Lb]
    wo = np.asarray(inputs["wo"], np.float64)[:Lb]
    w1 = np.asarray(inputs["w1"], np.float64)[:Lb]
    w2 = np.asarray(inputs["w2"], np.float64)[:Lb]
    bq = np.asarray(inputs["bq"], np.float64)[:Lb]
    bv = np.asarray(inputs["bv"], np.float64)[:Lb]
    bo = np.asarray(inputs["bo"], np.float64)[:Lb]
    b1 = np.asarray(inputs["b1"], np.float64)[:Lb]
    b2 = np.asarray(inputs["b2"], np.float64)[:Lb]
    fng = np.asarray(inputs["fn_g"], np.float64)
    fnb = np.asarray(inputs["fn_b"], np.float64)
    head_w = np.asarray(inputs["head_w"], np.float64)
    head_b = np.asarray(inputs["head_b"], np.float64)

    # fold LN1 gain/bias into Wq/Wk/Wv (+ 1/sqrt(dh) scale into Wq/bq);
    # drop K bias (softmax shift-invariance); fold V bias into bo via Wo.
    wq_f = g1[:, :, None] * wq * SCALE
    bq_f = (np.einsum("ld,ldf->lf", c1, wq) + bq) * SCALE
    wk_f = g1[:, :, None] * wk
    wv_f = g1[:, :, None] * wv
    bv_f = np.einsum("ld,ldf->lf", c1, wv) + bv
    bo_f = np.einsum("ld,ldf->lf", bv_f, wo) + bo
    # fold LN2 gain/bias into W1/b1
    w1_f = g2[:, :, None] * w1
    b1_f = np.einsum("ld,ldf->lf", c2, w1) + b1
    # fold final LN gain/bias into head
    hw_f = fng[:, None] * head_w
    hb_f = fnb @ head_w + head_b

    # bq: f = pair*128 + (h%2)*64 + dh -> sbuf [128, NPAIR]
    bq_pairs = f(bq_f.reshape(Lb, NPAIR, P).swapaxes(1, 2).astype(np.float32))
    b1_thirds = f(b1_f.reshape(Lb, FT, FCT, P).swapaxes(2, 3).astype(np.float32))

    base = {
        "masks": masks,
        "vones": np.ones((P, NKT, H, 1), bfloat16),
        "wq_s": bf(wq_f), "wk_s": bf(wk_f), "wv_s": bf(wv_f), "wo_s": bf(wo),
        "w1_s": bf(w1_f), "w2_s": bf(w2),
        "bq_s": bq_pairs,
        "bo_s": colmajor(bo_f),
        "b1_s": b1_thirds,
        "b2_s": colmajor(b2),
    }

    in_maps = []
    for c in range(NCORES):
        b = c // 4
        g = c % 4
        v0, vn = VSTART[g], VSLICE[g]
        hw_pad = np.zeros((D, VPAD), np.float64)
        hw_pad[:, :vn] = hw_f[:, v0:v0 + vn]
        x0 = tok_emb[tokens[b]] + pos_emb[:S]
        m = {"x0T": f(x0.T.astype(np.float32)), "hw_s": bf(hw_pad)}
        m.update(base)
        in_maps.append(m)
    return in_maps


def _get_nc():
    key = ("nc", L_BODY)
    if key not in _CACHE:
        _CACHE[key] = _build()
    return _CACHE[key]


def kernel(**inputs):
    nc = _get_nc()
    in_maps = _prep_inputs(inputs)
    res = bass_utils.run_bass_kernel_spmd(nc, in_maps, core_ids=list(range(NCORES)))
    # head bias is applied host-side (cheap; avoids a per-chunk broadcast
    # chain on-device)
    fnb = np.asarray(inputs["fn_b"], np.float64)
    head_w = np.asarray(inputs["head_w"], np.float64)
    hb_f = (fnb @ head_w + np.asarray(inputs["head_b"], np.float64)).astype(np.float32)
    out = np.empty((B, S, V), np.float32)
    for c in range(NCORES):
        b, g = c // 4, c % 4
        v0, vn = VSTART[g], VSLICE[g]
        out[b, :, v0:v0 + vn] = (
            np.asarray(res.results[c]["logits"])[:, :vn].astype(np.float32)
            + hb_f[v0:v0 + vn])
    return out


# revision 45
# speedup vs baseline: 1.9540x; 1.0058x over previous
"""GPT-2 small (L=12, D=768, H=12, S=1024, B=2, V=50257) forward pass on 8
Trainium2 NeuronCores via Bass/Tile.

Sharding: data-parallel over batch + vocab-parallel head, zero collectives
(AllReduce on this runtime costs ~150-250us fixed per call; 24 calls lose to
redundant compute). Cores 0-3 compute the full body for batch 0, cores 4-7
for batch 1; each core computes a quarter of the vocab head for its batch.

All matmuls run in bf16 (full PE rate at any moving-dim size, half the DMA
and SBUF of f32r), with f32 PSUM accumulation. Weights are host-folded:
  - LN1/LN2/final-LN gain+bias folded into Wq/Wk/Wv/W1/head_w and their
    biases (exact), so layernorm in-kernel is just (x-mean)*rstd.
  - K bias dropped (softmax is invariant to a per-query score shift).
  - V bias folded into the attention-output bias via bo' = bo + bv@Wo
    (softmax rows sum to 1).
  - The 1/sqrt(dh) scale is folded into Wq'/bq'.
Softmax denominator comes free out of the AV matmul: V tiles carry a 65th
ones-column per head, so PSUM row 64 accumulates sum(exp(scores)); exp uses
no max-subtraction (scores bounded for this model).
Layout: activations transposed (feature on partitions, tokens free), so all
dense matmuls take weights straight from HBM as stationary lhsT with no
transposes. Q/K projections are emitted per head-PAIR (stationary 128 wide).
FFN runs in 3 column-thirds with the residual accumulated directly into xT.
"""

import numpy as np
from ml_dtypes import bfloat16

import concourse.bass as bass
import concourse.tile as tile
from concourse import bacc, mybir
from concourse import bass_utils
from concourse.bass_isa import ReduceOp

F32 = mybir.dt.float32
BF16 = mybir.dt.bfloat16
AL = mybir.AluOpType
ACT = mybir.ActivationFunctionType

# model dims
B, S, D, H, DH, F4, V, L = 2, 1024, 768, 12, 64, 3072, 50257, 12
P = 128
KT = D // P            # 6 k-tiles over the model dim
EPS = 1e-5
SCALE = 1.0 / np.sqrt(DH)

# sharding / tiling
NCORES = 8
QB = 512               # query block
NQB = S // QB
NKT = S // P           # key tiles
NPAIR = H // 2         # head pairs
FT = 3                 # ffn thirds
FFC = F4 // FT         # 1024 ffn cols per third
FCT = FFC // P         # 8 fc tiles per third
VC = 512               # vocab chunk
VPAD = 12800           # padded per-core vocab slice (25 chunks of 512)
NVC = VPAD // VC
VSLICE = [12565, 12564, 12564, 12564]
VSTART = [0, 12565, 25129, 37693]

L_BODY = L  # overridable before first kernel() call for debugging

_CACHE = {}


def _build():
    nc = bacc.Bacc("TRN2", target_bir_lowering=False, debug=False,
                   num_devices=NCORES)

    # register EPS as a const AP so activation(bias=EPS) resolves
    eps_tensor = nc.alloc_sbuf_tensor(f"const-float32-{EPS}", [P, 1], F32)
    nc.gpsimd.memset(eps_tensor.ap(), EPS)
    nc.const_aps.aps[(F32, EPS)] = eps_tensor.ap()

    def di(name, shape, dt=F32):
        return nc.dram_tensor(name, shape, dt, kind="ExternalInput").ap()

    x0T = di("x0T", [D, S])
    masks = di("masks", [P, NKT // 2, QB], BF16)
    vones = di("vones", [P, NKT, H, 1], BF16)
    wq_s = di("wq_s", [L_BODY, D, D], BF16)
    wk_s = di("wk_s", [L_BODY, D, D], BF16)
    wv_s = di("wv_s", [L_BODY, D, D], BF16)
    wo_s = di("wo_s", [L_BODY, D, D], BF16)
    w1_s = di("w1_s", [L_BODY, D, F4], BF16)
    w2_s = di("w2_s", [L_BODY, F4, D], BF16)
    bq_s = di("bq_s", [L_BODY, P, NPAIR])
    bo_s = di("bo_s", [L_BODY, P, KT])
    b1_s = di("b1_s", [L_BODY, FT, P, FCT])
    b2_s = di("b2_s", [L_BODY, P, KT])
    hw_s = di("hw_s", [D, VPAD], BF16)
    logits = nc.dram_tensor("logits", [S, VPAD], BF16, kind="ExternalOutput").ap()

    with tile.TileContext(nc) as tc:
        with tc.tile_pool(name="persist", bufs=1) as persist, \
             tc.tile_pool(name="zp", bufs=2) as zpool, \
             tc.tile_pool(name="vp", bufs=1) as vpool, \
             tc.tile_pool(name="qk", bufs=2) as qkpool, \
             tc.tile_pool(name="op", bufs=2) as opool, \
             tc.tile_pool(name="wp", bufs=1) as wpool, \
             tc.tile_pool(name="fp", bufs=2) as fpool, \
             tc.tile_pool(name="ffp", bufs=2) as ffpool, \
             tc.tile_pool(name="ep", bufs=3) as eppool, \
             tc.tile_pool(name="sums", bufs=2) as sums, \
             tc.tile_pool(name="tmp", bufs=1) as tmp, \
             tc.tile_pool(name="small", bufs=2) as small, \
             tc.tile_pool(name="psA", bufs=5, space="PSUM") as psA, \
             tc.tile_pool(name="psO", bufs=3, space="PSUM") as psO:

            xT = persist.tile([P, KT, S], F32)
            nc.sync.dma_start(xT, x0T.rearrange("(t p) q -> p t q", p=P))
            masks_sb = persist.tile([P, NKT // 2, QB], BF16)
            nc.sync.dma_start(masks_sb, masks)

            # V tiles: per head 64 value dims + a 65th ones column so the AV
            # matmul accumulates sum(exp) into PSUM row 64. Written once.
            v_t = vpool.tile([P, NKT, H, 65], BF16)
            nc.sync.dma_start(v_t[:, :, :, 64:65], vones)

            def ln_stat_tiles():
                st = tmp.tile([P, 2, QB], F32, tag="st")
                return st

            def ln_stat_oc(st, qb, oc):
                """Accumulate LN sums for one feature tile of xT, emitted
                right after that tile's residual add so the stats pipeline
                with the producing matmul loop. st[:,0]=sum, st[:,1]=sumsq."""
                qs = slice(qb * QB, (qb + 1) * QB)
                tsq = tmp.tile([P, QB], F32, tag="t3")
                if oc == 0:
                    nc.vector.tensor_copy(st[:, 0, :], xT[:, 0, qs])
                    nc.scalar.activation(st[:, 1, :], xT[:, 0, qs], ACT.Square)
                else:
                    nc.vector.tensor_tensor(st[:, 0, :], st[:, 0, :],
                                            xT[:, oc, qs], AL.add)
                    nc.scalar.activation(tsq, xT[:, oc, qs], ACT.Square)
                    nc.vector.tensor_tensor(st[:, 1, :], st[:, 1, :], tsq, AL.add)

            def ln_finish(st, z, qb):
                """(x - mean) * rstd from accumulated sums. Gain/bias are
                folded into downstream weights on the host. The normalize is
                split across DVE and Pool to halve its serial tail."""
                qs = slice(qb * QB, (qb + 1) * QB)
                t1 = tmp.tile([P, QB], F32, tag="t1")
                t2 = tmp.tile([P, QB], F32, tag="t2")
                t3 = tmp.tile([P, QB], F32, tag="t3")
                t4 = tmp.tile([P, QB], F32, tag="t4")
                # one fused partition reduce for both sums
                nc.gpsimd.partition_all_reduce(st, st, P, ReduceOp.add)
                # t1 = mean^2 * D = (acc/D)*acc
                nc.vector.scalar_tensor_tensor(
                    t1, st[:, 0, :], 1.0 / D, st[:, 0, :], AL.mult, AL.mult)
                nc.vector.tensor_tensor(t1, st[:, 1, :], t1, AL.subtract)
                # sigma = sqrt((asq - m^2 D)/D + eps), then rstd = 1/sigma
                nc.scalar.activation(t1, t1, ACT.Sqrt, scale=1.0 / D, bias=EPS)
                nc.vector.reciprocal(t1, t1)                          # rstd
                # t2 = mean * rstd
                nc.vector.scalar_tensor_tensor(
                    t2, st[:, 0, :], 1.0 / D, t1, AL.mult, AL.mult)
                for kt in range(4):
                    nc.vector.tensor_tensor(t3, xT[:, kt, qs], t1, AL.mult)
                    nc.vector.tensor_tensor(z[:, kt, qs], t3, t2, AL.subtract)
                for kt in range(4, KT):
                    nc.gpsimd.tensor_tensor(t4, xT[:, kt, qs], t1, AL.mult)
                    nc.gpsimd.tensor_tensor(z[:, kt, qs], t4, t2, AL.subtract)

            def layer_norm():
                z = zpool.tile([P, KT, S], BF16, tag="z")
                for qb in range(NQB):
                    st = ln_stat_tiles()
                    for oc in range(KT):
                        ln_stat_oc(st, qb, oc)
                    ln_finish(st, z, qb)
                return z

            z = layer_norm()
            for l in range(L_BODY):
                # ---- attention ----
                wv_t = wpool.tile([P, KT, D], BF16, tag="wv")
                nc.sync.dma_start(wv_t, wv_s[l].rearrange("(t p) f -> p t f", p=P))
                wq_t = wpool.tile([P, KT, D], BF16, tag="wq")
                nc.sync.dma_start(wq_t, wq_s[l].rearrange("(t p) f -> p t f", p=P))
                wk_t = wpool.tile([P, KT, D], BF16, tag="wk")
                nc.sync.dma_start(wk_t, wk_s[l].rearrange("(t p) f -> p t f", p=P))
                wo_t = wpool.tile([P, KT, D], BF16, tag="wo")
                nc.sync.dma_start(wo_t, wo_s[l].rearrange("(t p) f -> p t f", p=P))
                bq_t = small.tile([P, NPAIR], F32, tag="bq")
                nc.sync.dma_start(bq_t, bq_s[l])

                # V projection: [keys, features] layout, 2 chunks of 384 cols
                def v_proj(tcs):
                    for tc_ in tcs:
                        for c2 in range(2):
                            ps = psA.tile([P, QB], F32, tag="psA")
                            for kt in range(KT):
                                nc.tensor.matmul(
                                    ps[:, :384], z[:, kt, tc_ * P:(tc_ + 1) * P],
                                    wv_t[:, kt, c2 * 384:(c2 + 1) * 384],
                                    start=kt == 0, stop=kt == KT - 1)
                            nc.vector.tensor_copy(
                                v_t[:, tc_, c2 * 6:(c2 + 1) * 6, 0:64],
                                ps[:, :384].rearrange("p (h d) -> p h d", d=DH))

                # keys 0-511 now; keys 512-1023 deferred until the first
                # pair's qb0 attention is emitted, so the PE has qb0-only
                # work while the layer-boundary LN of qb1 finishes
                v_proj(range(0, NKT // 2))

                oTs = []
                for _qb in range(NQB):
                    oT_b = opool.tile([P, KT, QB], BF16, tag="o", name=f"oT{_qb}")
                    oTs.append(oT_b)

                def qk_proj(pair, qp, kp, qb):
                    qs = slice(qb * QB, (qb + 1) * QB)
                    psq = psA.tile([P, QB], F32, tag="psA")
                    for kt in range(KT):
                        nc.tensor.matmul(
                            psq, wq_t[:, kt, pair * P:(pair + 1) * P],
                            z[:, kt, qs], start=kt == 0, stop=kt == KT - 1)
                    nc.vector.tensor_scalar_add(
                        qp[:, qs], psq, bq_t[:, pair:pair + 1])
                    psk = psA.tile([P, QB], F32, tag="psA")
                    for kt in range(KT):
                        nc.tensor.matmul(
                            psk, wk_t[:, kt, pair * P:(pair + 1) * P],
                            z[:, kt, qs], start=kt == 0, stop=kt == KT - 1)
                    nc.vector.tensor_copy(kp[:, qs], psk)

                def attn_unit(pair, qp, kp, qb):
                    qs = slice(qb * QB, (qb + 1) * QB)
                    nkt = 4 * qb + 4
                    o_ps = []
                    for h2 in range(2):
                        ops_h = psO.tile([65, QB], F32, tag="psO",
                                         name=f"ops{h2}")
                        o_ps.append(ops_h)
                    # software pipeline: AV of tile ti-1 is emitted after
                    # the scores of tile ti, so exp/mask latency is hidden
                    prev = None
                    for ti in range(nkt):
                        cur = []
                        for h2 in range(2):
                            po = h2 * DH
                            s_ps = psA.tile([P, QB], F32, tag="psA")
                            nc.tensor.matmul(
                                s_ps, kp[po:po + DH, ti * P:(ti + 1) * P],
                                qp[po:po + DH, qs], start=True, stop=True)
                            e_t = eppool.tile([P, QB], BF16, tag="e", bufs=5)
                            nc.scalar.activation(e_t, s_ps, ACT.Exp)
                            r = ti - 4 * qb
                            if r >= 0:
                                nc.vector.tensor_tensor(
                                    e_t, e_t, masks_sb[:, r, :], AL.mult)
                            cur.append(e_t)
                        if prev is not None:
                            for h2 in range(2):
                                nc.tensor.matmul(
                                    o_ps[h2], v_t[:, ti - 1, 2 * pair + h2, :],
                                    prev[h2], start=ti - 1 == 0, stop=False)
                        prev = cur
                    for h2 in range(2):
                        nc.tensor.matmul(
                            o_ps[h2], v_t[:, nkt - 1, 2 * pair + h2, :],
                            prev[h2], start=nkt == 1, stop=True)
                    for h2 in range(2):
                        po = h2 * DH
                        rcp = sums.tile([DH, QB], F32, tag="rcp", bufs=1)
                        nc.vector.reciprocal(rcp[0:1, :], o_ps[h2][64:65, :])
                        nc.gpsimd.partition_broadcast(rcp, rcp[0:1, :])
                        nc.vector.tensor_tensor(
                            oTs[qb][po:po + DH, pair, :],
                            o_ps[h2][0:64, :], rcp, AL.mult)

                # pair-level software pipeline: projections of pair p are
                # emitted before the attention of pair p-1, so the psum->sbuf
                # copies always have a full projection block of PE work to
                # complete behind
                pk_prev = None
                for pair in range(NPAIR):
                    qp = qkpool.tile([P, S], BF16, tag="qp")
                    kp = qkpool.tile([P, S], BF16, tag="kp")
                    qk_proj(pair, qp, kp, 0)
                    qk_proj(pair, qp, kp, 1)
                    if pair == 0:
                        v_proj(range(NKT // 2, NKT))
                    if pk_prev is not None:
                        pp, pqp, pkp = pk_prev
                        attn_unit(pp, pqp, pkp, 0)
                        attn_unit(pp, pqp, pkp, 1)
                    pk_prev = (pair, qp, kp)
                pp, pqp, pkp = pk_prev
                attn_unit(pp, pqp, pkp, 0)
                attn_unit(pp, pqp, pkp, 1)

                bo_t = small.tile([P, KT], F32, tag="bias")
                nc.sync.dma_start(bo_t, bo_s[l])
                z2 = zpool.tile([P, KT, S], BF16, tag="z", name="z2")
                for qb in range(NQB):
                    qs = slice(qb * QB, (qb + 1) * QB)
                    st = ln_stat_tiles()
                    for oc in range(KT):
                        ps = psA.tile([P, QB], F32, tag="psA")
                        for kt in range(KT):
                            nc.tensor.matmul(
                                ps, wo_t[:, kt, oc * P:(oc + 1) * P],
                                oTs[qb][:, kt, :], start=kt == 0, stop=kt == KT - 1)
                        nc.vector.tensor_tensor(xT[:, oc, qs], xT[:, oc, qs], ps, AL.add)
                        nc.vector.tensor_scalar_add(
                            xT[:, oc, qs], xT[:, oc, qs], bo_t[:, oc:oc + 1])
                        # LN2 stats pipeline with the O-proj oc loop
                        ln_stat_oc(st, qb, oc)
                    ln_finish(st, z2, qb)

                # ---- ffn (qb outer so LN of the next layer overlaps) ----
                b2_t = small.tile([P, KT], F32, tag="bias")
                nc.sync.dma_start(b2_t, b2_s[l])
                z_next = zpool.tile([P, KT, S], BF16, tag="z", name="z_next")
                for qb in range(NQB):
                    qs = slice(qb * QB, (qb + 1) * QB)
                    for g in range(FT):
                        w1_t = fpool.tile([P, KT, FFC], BF16, tag="w1")
                        nc.sync.dma_start(
                            w1_t, w1_s[l][:, g * FFC:(g + 1) * FFC]
                            .rearrange("(t p) f -> p t f", p=P))
                        w2_t = fpool.tile([P, FCT, D], BF16, tag="w2")
                        nc.sync.dma_start(
                            w2_t, w2_s[l][g * FFC:(g + 1) * FFC, :]
                            .rearrange("(t p) f -> p t f", p=P))
                        b1_t = small.tile([P, FCT], F32, tag="b1")
                        nc.sync.dma_start(b1_t, b1_s[l, g])
                        ffT = ffpool.tile([P, FCT, QB], BF16, tag="ff")
                        for fc in range(FCT):
                            ps = psA.tile([P, QB], F32, tag="psA")
                            for kt in range(KT):
                                nc.tensor.matmul(
                                    ps, w1_t[:, kt, fc * P:(fc + 1) * P],
                                    z2[:, kt, qs], start=kt == 0, stop=kt == KT - 1)
                            nc.scalar.activation(
                                ffT[:, fc, :], ps, ACT.Gelu,
                                bias=b1_t[:, fc:fc + 1])
                        st = ln_stat_tiles() if g == FT - 1 else None
                        for oc in range(KT):
                            ps = psA.tile([P, QB], F32, tag="psA")
                            for kt in range(FCT):
                                nc.tensor.matmul(
                                    ps, w2_t[:, kt, oc * P:(oc + 1) * P],
                                    ffT[:, kt, :], start=kt == 0, stop=kt == FCT - 1)
                            nc.vector.tensor_tensor(
                                xT[:, oc, qs], xT[:, oc, qs], ps, AL.add)
                            if g == FT - 1:
                                nc.vector.tensor_scalar_add(
                                    xT[:, oc, qs], xT[:, oc, qs], b2_t[:, oc:oc + 1])
                                # next layer's LN stats pipeline with FFN2
                                ln_stat_oc(st, qb, oc)
                    # LN for the next layer (or the folded final LN) for this
                    # query block, overlapped with the other block's FFN
                    ln_finish(st, z_next, qb)
                z = z_next

            # ---- vocab head (final LN folded: z is the final-LN output;
            # head bias is added on the host) ----
            zf = z
            for vc in range(NVC):
                vs = slice(vc * VC, (vc + 1) * VC)
                hw_t = fpool.tile([P, KT, VC], BF16, tag="w1")
                nc.sync.dma_start(hw_t, hw_s[:, vs].rearrange("(t p) v -> p t v", p=P))
                for tc_ in range(NKT):
                    ps = psA.tile([P, QB], F32, tag="psA")
                    for kt in range(KT):
                        nc.tensor.matmul(
                            ps[:, :VC], zf[:, kt, tc_ * P:(tc_ + 1) * P],
                            hw_t[:, kt, :], start=kt == 0, stop=kt == KT - 1)
                    lg = eppool.tile([P, VC], BF16, tag="lg", bufs=3)
                    nc.scalar.activation(lg, ps[:, :VC], ACT.Copy)
                    # Pool-issued DMA: keeps logits writes off the SP queue so
                    # they don't head-of-line-block the next hw_t load
                    nc.gpsimd.dma_start(logits[tc_ * P:(tc_ + 1) * P, vs], lg)

    nc.finalize()
    return nc


def _prep_inputs(inputs):
    f = np.ascontiguousarray
    tokens = np.asarray(inputs["tokens"])
    tok_emb = np.asarray(inputs["tok_emb"], np.float32)
    pos_emb = np.asarray(inputs["pos_emb"], np.float32)

    Lb = L_BODY

    def colmajor(a):  # [..., D] -> [..., P, KT] per-partition columns
        return f(a.reshape(*a.shape[:-1], KT, P).swapaxes(-1, -2).astype(np.float32))

    def bf(a):
        return f(np.asarray(a).astype(bfloat16))

    masks = (np.arange(P)[:, None, None] + P * np.arange(NKT // 2)[None, :, None]
             <= np.arange(QB)[None, None, :]).astype(bfloat16)

    g1 = np.asarray(inputs["ln1_g"], np.float64)[:Lb]   # [L, D]
    c1 = np.asarray(inputs["ln1_b"], np.float64)[:Lb]
    g2 = np.asarray(inputs["ln2_g"], np.float64)[:Lb]
    c2 = np.asarray(inputs["ln2_b"], np.float64)[:Lb]
    wq = np.asarray(inputs["wq"], np.float64)[:Lb]      # [L, D, D]
    wk = np.asarray(inputs["wk"], np.float64)[:Lb]
    wv = np.asarray(inputs["wv"], np.float64)[:Lb]
    wo = np.asarray(inputs["wo"], np.float64)[:Lb]
    w1 = np.asarray(inputs["w1"], np.float64)[:Lb]
    w2 = np.asarray(inputs["w2"], np.float64)[:Lb]
    bq = np.asarray(inputs["bq"], np.float64)[:Lb]
    bv = np.asarray(inputs["bv"], np.float64)[:Lb]
    bo = np.asarray(inputs["bo"], np.float64)[:Lb]
    b1 = np.asarray(inputs["b1"], np.float64)[:Lb]
    b2 = np.asarray(inputs["b2"], np.float64)[:Lb]
    fng = np.asarray(inputs["fn_g"], np.float64)
    fnb = np.asarray(inputs["fn_b"], np.float64)
    head_w = np.asarray(inputs["head_w"], np.float64)
    head_b = np.asarray(inputs["head_b"], np.float64)

    # fold LN1 gain/bias into Wq/Wk/Wv (+ 1/sqrt(dh) scale into Wq/bq);
    # drop K bias (softmax shift-invariance); fold V bias into bo via Wo.
    wq_f = g1[:, :, None] * wq * SCALE
    bq_f = (np.einsum("ld,ldf->lf", c1, wq) + bq) * SCALE
    wk_f = g1[:, :, None] * wk
    wv_f = g1[:, :, None] * wv
    bv_f = np.einsum("ld,ldf->lf", c1, wv) + bv
    bo_f = np.einsum("ld,ldf->lf", bv_f, wo) + bo
    # fold LN2 gain/bias into W1/b1
    w1_f = g2[:, :, None] * w1
    b1_f = np.einsum("ld,ldf->lf", c2, w1) + b1
    # fold final LN gain/bias into head
    hw_f = fng[:, None] * head_w
    hb_f = fnb @ head_w + head_b

    # bq: f = pair*128 + (h%2)*64 + dh -> sbuf [128, NPAIR]
    bq_pairs = f(bq_f.reshape(Lb, NPAIR, P).swapaxes(1, 2).astype(np.float32))
    b1_thirds = f(b1_f.reshape(Lb, FT, FCT, P).swapaxes(2, 3).astype(np.float32))

    base = {
        "masks": masks,
        "vones": np.ones((P, NKT, H, 1), bfloat16),
        "wq_s": bf(wq_f), "wk_s": bf(wk_f), "wv_s": bf(wv_f), "wo_s": bf(wo),
        "w1_s": bf(w1_f), "w2_s": bf(w2),
        "bq_s": bq_pairs,
        "bo_s": colmajor(bo_f),
        "b1_s": b1_thirds,
        "b2_s": colmajor(b2),
    }

    in_maps = []
    for c in range(NCORES):
        b = c // 4
        g = c % 4
        v0, vn = VSTART[g], VSLICE[g]
        hw_pad = np.zeros((D, VPAD), np.float64)
        hw_pad[:, :vn] = hw_f[:, v0:v0 + vn]
        x0 = tok_emb[tokens[b]] + pos_emb[:S]
        m = {"x0T": f(x0.T.astype(np.float32)), "hw_s": bf(hw_pad)}
        m.update(base)
        in_maps.append(m)
    return in_maps


def _get_nc():
    key = ("nc", L_BODY)
    if key not in _CACHE:
        _CACHE[key] = _build()
    return _CACHE[key]


def kernel(**inputs):
    nc = _get_nc()
    in_maps = _prep_inputs(inputs)
    res = bass_utils.run_bass_kernel_spmd(nc, in_maps, core_ids=list(range(NCORES)))
    # head bias is applied host-side (cheap; avoids a per-chunk broadcast
    # chain on-device)
    fnb = np.asarray(inputs["fn_b"], np.float64)
    head_w = np.asarray(inputs["head_w"], np.float64)
    hb_f = (fnb @ head_w + np.asarray(inputs["head_b"], np.float64)).astype(np.float32)
    out = np.empty((B, S, V), np.float32)
    for c in range(NCORES):
        b, g = c // 4, c % 4
        v0, vn = VSTART[g], VSLICE[g]
        out[b, :, v0:v0 + vn] = (
            np.asarray(res.results[c]["logits"])[:, :vn].astype(np.float32)
            + hb_f[v0:v0 + vn])
    return out
